# revision 1
# baseline (speedup 1.0000x reference)
"""Trainium2 Bass kernel for CardRecommendationSAGE (2-layer mean-aggregate SAGEConv + linear head).

Strategy (8 NeuronCores, SPMD, single NEFF):
  - Nodes are sharded into 8 contiguous blocks of 12544 (N padded 100000->100352).
    Edges are partitioned by destination core, so segment-sum is local.
  - Everything on-chip is kept feature-major ([feat, nodes]); weights are
    pre-transposed on the host so no on-device transposes are needed.
  - Layer-1 neighbor projection p1 = h0 @ Wl1.T is computed for ALL nodes on
    every core (x is replicated) -- this removes one AllGather entirely.
  - mean-aggregation is a matmul: for each 128-edge tile, a selection matrix
    S[e,d] = (dst_local[e]==d) * deg_inv[dst[e]] is built on the vector engine
    with a single dual-op tensor_scalar (is_equal then mult), and
    PSUM += P_gathered^T @ S accumulates the neighbor means per 128-node window.
  - Edge-source rows are fetched with the bulk dma_gather custom op (int16
    indices, so the node space is split into 4 chunks of 25088 rows).
  - Layer 2 needs one AllGather of p2 = h1 @ Wl2.T; it is split into 7 slab
    AllGathers so communication overlaps the tail of layer-1 compute.
  - p1/p2_full live in a slab-permuted layout: node (c,r) -> slab j=r//1792,
    row j*14336 + c*1792 + r%1792, which makes each slab-AllGather write one
    contiguous region while both layers share the same gather indices.
"""

import math
import os
import numpy as np

# ---------------- problem constants (hardcoded; kernel must be self-contained)
N = 100000
F = 128          # node feature dim (plus 1 cards indicator)
H = 128
C = 110          # num cards
NCORE = 8
NB = 12544       # nodes per core
NPAD = NCORE * NB  # 100352
WPC = NB // 128    # 98 windows per core
WTOT = NPAD // 128  # 784
SLAB_W = 14        # windows per AllGather slab (98 = 7*14)
NSLAB = 7
SLAB_ROWS = SLAB_W * 128        # 1792 rows per core per slab
SLAB_TOT = NCORE * SLAB_ROWS    # 14336 rows per slab
NCHUNK = 4
CHUNK_ROWS = NPAD // NCHUNK     # 25088 (< 2^15 so int16 indices work)
PJ = 7             # windows per stage-B group (784 = 112*7; 7 stays in one slab)
GW = 4             # windows per aggregation group

_F32 = np.float32

# wconst column map
WC_WL1 = 0
WC_WR1 = 128
WC_WL2 = 256
WC_WR2 = 384
WC_WO = 512          # [128,110]
WC_IOTA = 622
WC_WL1C = 750        # row 0 only, 128 cols
WC_WR1C = 878        # row 0 only, 128 cols
WC_B1 = 1006
WC_B2 = 1007
WC_BO = 1008         # partitions 0..109
WC_COLS = 1009


def _perm_row(n):
    """node id -> permuted row in p1/p2_full (slab layout)."""
    c, r = n // NB, n % NB
    j, rr = r // SLAB_ROWS, r % SLAB_ROWS
    return j * SLAB_TOT + c * SLAB_ROWS + rr


def _prep(x, Wl1, bl1, Wr1, br1, Wl2, bl2, Wr2, br2, Wo, bo, edge_index, input_cards):
    """Host-side sharding/preprocessing. Returns (in_maps, plan)."""
    E = edge_index.shape[1]
    xpad = np.zeros((NPAD, F), _F32)
    xpad[:N] = x
    cards_pad = np.zeros((NPAD,), _F32)
    cards_pad[:N] = input_cards.astype(_F32)
    xT_full = np.ascontiguousarray(xpad.T)           # [128, NPAD]

    src = edge_index[0].astype(np.int64)
    dst = edge_index[1].astype(np.int64)

    deg = np.bincount(dst, minlength=N).astype(_F32)
    deginv_node = np.zeros((NPAD,), _F32)
    nz = deg > 0
    deginv_node[:N][nz] = (_F32(1.0) / deg[nz]).astype(_F32)

    core = dst // NB
    wloc = (dst % NB) // 128
    dloc = dst % 128
    # permuted source rows for gathering
    psrc = (src % NB) // SLAB_ROWS * SLAB_TOT + (src // NB) * SLAB_ROWS + (src % NB) % SLAB_ROWS
    chunk = psrc // CHUNK_ROWS
    idx_local = psrc % CHUNK_ROWS

    key = ((core * WPC + wloc) * NCHUNK + chunk).astype(np.int64)
    cnt = np.bincount(key, minlength=NCORE * WPC * NCHUNK).reshape(NCORE, WPC, NCHUNK)
    Twc = np.ceil(cnt.max(axis=0) / 128).astype(np.int64)  # [WPC, NCHUNK]

    # tile layout order: for each group g of GW windows, for each chunk, windows in g
    groups = [list(range(s, min(s + GW, WPC))) for s in range(0, WPC, GW)]
    run_off = np.zeros((WPC, NCHUNK), np.int64)  # tile offset of run (w,c)
    calls = []  # (chunk, tile_start, ntiles, group_index)
    tcount = 0
    for gi, ws in enumerate(groups):
        for c in range(NCHUNK):
            ts = tcount
            for w in ws:
                run_off[w, c] = tcount
                tcount += Twc[w, c]
            if tcount > ts:
                calls.append((c, ts, int(tcount - ts), gi))
    T_total = int(tcount)
    S_slots = T_total * 128

    # per-edge slot
    order = np.argsort(key, kind="stable")
    ks = key[order]
    first = np.zeros(NCORE * WPC * NCHUNK + 1, np.int64)
    np.cumsum(np.bincount(ks, minlength=NCORE * WPC * NCHUNK), out=first[1:])
    within = np.arange(E, dtype=np.int64) - first[ks]
    run_off_of_edge = run_off.reshape(-1)[(ks % (WPC * NCHUNK))]
    slot_sorted = run_off_of_edge * 128 + within
    slot = np.empty(E, np.int64)
    slot[order] = slot_sorted

    # weights const block
    wconst = np.zeros((128, WC_COLS), _F32)
    wconst[:, WC_WL1:WC_WL1 + 128] = Wl1[:, :F].T
    wconst[:, WC_WR1:WC_WR1 + 128] = Wr1[:, :F].T
    wconst[:, WC_WL2:WC_WL2 + 128] = Wl2.T
    wconst[:, WC_WR2:WC_WR2 + 128] = Wr2.T
    wconst[:, WC_WO:WC_WO + C] = Wo.T
    wconst[:, WC_IOTA:WC_IOTA + 128] = np.arange(128, dtype=_F32)[None, :]
    wconst[0, WC_WL1C:WC_WL1C + 128] = Wl1[:, F]
    wconst[0, WC_WR1C:WC_WR1C + 128] = Wr1[:, F]
    wconst[:, WC_B1] = bl1 + br1
    wconst[:, WC_B2] = bl2 + br2
    wconst[:C, WC_BO] = bo

    in_maps = []
    for c8 in range(NCORE):
        m = core == c8
        sl = slot[m]
        dstl = np.full(S_slots, 999.0, _F32)
        dstl[sl] = dloc[m].astype(_F32)
        dginv = np.zeros(S_slots, _F32)
        dginv[sl] = deginv_node[dst[m]]
        idxl = np.zeros(S_slots, np.int64)
        idxl[sl] = idx_local[m]

        dstl_sb = np.ascontiguousarray(dstl.reshape(T_total, 128).T)
        dginv_sb = np.ascontiguousarray(dginv.reshape(T_total, 128).T)
        i16 = idxl.reshape(T_total * 8, 16).T.astype(np.int16)  # [16, T*8]
        idx16 = np.ascontiguousarray(np.tile(i16, (8, 1)))      # [128, T*8]

        blk = slice(c8 * NB, (c8 + 1) * NB)
        in_maps.append(dict(
            xT_full=xT_full,
            xT_own=np.ascontiguousarray(xpad[blk].T),
            cards_all=cards_pad[None, :],
            cards_own=np.ascontiguousarray(cards_pad[None, blk]),
            idx16=idx16,
            dstl=dstl_sb,
            dginv=dginv_sb,
            wconst=wconst,
        ))

    plan = dict(Twc=Twc, run_off=run_off, calls=calls, groups=groups, T_total=T_total)
    return in_maps, plan


def _build(plan):
    import concourse.bacc as bacc
    import concourse.bass as bass
    import concourse.mybir as mybir
    import concourse.tile as tile
    from concourse.tile_rust import add_dep_helper

    Twc = plan["Twc"]
    run_off = plan["run_off"]
    calls = plan["calls"]
    groups = plan["groups"]
    T_total = plan["T_total"]
    f32 = mybir.dt.float32

    nc = bacc.Bacc(num_devices=NCORE)

    xT_full = nc.dram_tensor("xT_full", [128, NPAD], f32, kind="ExternalInput")
    xT_own = nc.dram_tensor("xT_own", [128, NB], f32, kind="ExternalInput")
    cards_all = nc.dram_tensor("cards_all", [1, NPAD], f32, kind="ExternalInput")
    cards_own = nc.dram_tensor("cards_own", [1, NB], f32, kind="ExternalInput")
    idx16 = nc.dram_tensor("idx16", [128, T_total * 8], mybir.dt.int16, kind="ExternalInput")
    dstl = nc.dram_tensor("dstl", [128, T_total], f32, kind="ExternalInput")
    dginv = nc.dram_tensor("dginv", [128, T_total], f32, kind="ExternalInput")
    wconst = nc.dram_tensor("wconst", [128, WC_COLS], f32, kind="ExternalInput")

    logitsT = nc.dram_tensor("logitsT", [C, NB], f32, kind="ExternalOutput")

    p1 = nc.dram_tensor("p1_full", [NPAD, F], f32)
    p2own = nc.dram_tensor("p2_own", [NB, F], f32)
    p2full = nc.dram_tensor("p2_full", [NPAD, F], f32, addr_space="Shared")
    p2loc = nc.dram_tensor("p2_loc", [NPAD, F], f32)

    max_call_tiles = max(nt for (_, _, nt, _) in calls)

    with tile.TileContext(nc) as tc:
        with (
            tc.tile_pool(name="const", bufs=1) as cpool,
            tc.tile_pool(name="xs", bufs=3) as xs_pool,
            tc.tile_pool(name="cb", bufs=2) as cb_pool,
            tc.tile_pool(name="p1s", bufs=3) as p1s_pool,
            tc.tile_pool(name="gb", bufs=3) as gb_pool,
            tc.tile_pool(name="ixs", bufs=4) as ixs_pool,
            tc.tile_pool(name="st", bufs=6) as st_pool,
            tc.tile_pool(name="cg", bufs=2) as cg_pool,
            tc.tile_pool(name="h2", bufs=3) as h2_pool,
            tc.tile_pool(name="lg", bufs=3) as lg_pool,
            tc.tile_pool(name="psB", bufs=2, space="PSUM") as psB_pool,
            tc.tile_pool(name="psA", bufs=4, space="PSUM") as psA_pool,
            tc.tile_pool(name="psL", bufs=2, space="PSUM") as psL_pool,
        ):
            # ---- resident constants
            wc = cpool.tile([128, WC_COLS], f32, tag="wconst")
            nc.sync.dma_start(out=wc[:], in_=wconst[:])
            xo = cpool.tile([128, NB], f32, tag="xT_own")
            nc.sync.dma_start(out=xo[:], in_=xT_own[:])
            dl = cpool.tile([128, T_total], f32, tag="dstl")
            nc.sync.dma_start(out=dl[:], in_=dstl[:])
            dg = cpool.tile([128, T_total], f32, tag="dginv")
            nc.sync.dma_start(out=dg[:], in_=dginv[:])
            h1T = cpool.tile([128, NB], f32, tag="h1T")

            iota = wc[:, WC_IOTA:WC_IOTA + 128]

            # ---- stage B: p1 for ALL nodes (x replicated), written to slab layout
            p1_writes_by_chunk = {k: [] for k in range(NCHUNK)}
            NBG = WTOT // PJ  # 112
            def _pstart_of(mg):
                w = mg * PJ
                cg_, wl = w // WPC, w % WPC
                return (wl // SLAB_W) * SLAB_TOT + cg_ * SLAB_ROWS + (wl % SLAB_W) * 128
            for mg in sorted(range(NBG), key=_pstart_of):
                w0 = mg * PJ
                xs = xs_pool.tile([128, PJ * 128], f32, tag="xs")
                nc.sync.dma_start(out=xs[:], in_=xT_full[:, w0 * 128:(w0 + PJ) * 128])
                cb = cb_pool.tile([1, PJ * 128], f32, tag="cb")
                nc.sync.dma_start(out=cb[:], in_=cards_all[0:1, w0 * 128:(w0 + PJ) * 128])
                p1s = p1s_pool.tile([128, PJ * 128], f32, tag="p1s")
                for k in range(PJ):
                    ps = psB_pool.tile([128, 128], f32, space="PSUM", tag="psB")
                    nc.tensor.matmul(out=ps[:], lhsT=xs[:, k * 128:(k + 1) * 128],
                                     rhs=wc[:, WC_WL1:WC_WL1 + 128], start=True, stop=False)
                    nc.tensor.matmul(out=ps[:], lhsT=cb[0:1, k * 128:(k + 1) * 128],
                                     rhs=wc[0:1, WC_WL1C:WC_WL1C + 128], start=False, stop=True)
                    nc.scalar.activation(out=p1s[:, k * 128:(k + 1) * 128], in_=ps[:],
                                         func=mybir.ActivationFunctionType.Copy)
                # perm dest: windows w0..w0+6 are consecutive within one slab run
                w = w0
                cg_, wl = w // WPC, w % WPC
                pstart = (wl // SLAB_W) * SLAB_TOT + cg_ * SLAB_ROWS + (wl % SLAB_W) * 128
                wi = nc.sync.dma_start(
                    out=p1[pstart:pstart + PJ * 128, :].rearrange("(k p) f -> p k f", p=128),
                    in_=p1s[:].rearrange("p (k f) -> p k f", f=128),
                )
                p1_writes_by_chunk[pstart // CHUNK_ROWS].append(wi)

            # ---- aggregation stage builder (shared by both layers)
            def aggregate(layer, src_dram, gather_dep_insts):
                """layer 1: self from xT_own(+cards), out h1T slices.
                   layer 2: self from h1T, out h2 tiles -> logits."""
                for gi, ws in enumerate(groups):
                    gbufs = {}
                    for (c, ts, nt, g2) in calls:
                        if g2 != gi:
                            continue
                        gb = gb_pool.tile([128, max_call_tiles * 128], f32, tag="gb")
                        ixt = ixs_pool.tile([128, max_call_tiles * 8], mybir.dt.int16, tag="ixs")
                        nc.sync.dma_start(out=ixt[:, :nt * 8], in_=idx16[:, 8 * ts:8 * (ts + nt)])
                        # single_packet=True faults the device above ~1024 indices per
                        # call; with single_packet=False one bulk call per (group, chunk)
                        # is fine and amortizes the ~1us SWDGE fixed cost.
                        gcall = nc.gpsimd.dma_gather(
                            gb[:, :nt * 128].rearrange("p (t e) -> p t e", e=128),
                            src_dram[c * CHUNK_ROWS:(c + 1) * CHUNK_ROWS, :],
                            ixt[:, :nt * 8],
                            nt * 128,
                            nt * 128,
                            128,
                            single_packet=False,
                        )
                        for dep in gather_dep_insts(c):
                            add_dep_helper(gcall.ins, dep.ins, True, "dram raw")
                        gbufs[c] = (gb, ts)
                    if layer == 1:
                        cg = cg_pool.tile([1, GW * 128], f32, tag="cg")
                        nc.sync.dma_start(
                            out=cg[0:1, :len(ws) * 128],
                            in_=cards_own[0:1, ws[0] * 128:(ws[-1] + 1) * 128])
                    pss = {}
                    last_mm = {}  # w -> (c, t) of final neighbor matmul
                    for w in ws:
                        nb_tiles = [(c, t) for c in range(NCHUNK) for t in range(Twc[w, c])]
                        last_mm[w] = nb_tiles[-1] if nb_tiles else None
                        ps = psA_pool.tile([128, 128], f32, space="PSUM", tag="psA")
                        pss[w] = ps
                        if layer == 1:
                            nc.tensor.matmul(out=ps[:], lhsT=wc[:, WC_WR1:WC_WR1 + 128],
                                             rhs=xo[:, w * 128:(w + 1) * 128],
                                             start=True, stop=False)
                            lo = (w - ws[0]) * 128
                            nc.tensor.matmul(out=ps[:], lhsT=wc[0:1, WC_WR1C:WC_WR1C + 128],
                                             rhs=cg[0:1, lo:lo + 128],
                                             start=False, stop=(last_mm[w] is None))
                        else:
                            nc.tensor.matmul(out=ps[:], lhsT=wc[:, WC_WR2:WC_WR2 + 128],
                                             rhs=h1T[:, w * 128:(w + 1) * 128],
                                             start=True, stop=(last_mm[w] is None))
                    st_i = 0
                    for c in range(NCHUNK):
                        if c not in gbufs:
                            continue
                        gb, ts = gbufs[c]
                        for w in ws:
                            for t in range(Twc[w, c]):
                                col = run_off[w, c] + t
                                lc = col - ts
                                st = st_pool.tile([128, 128], f32, tag="st")
                                st_i += 1
                                nc.vector.tensor_scalar(
                                    out=st[:], in0=iota,
                                    scalar1=dl[:, col:col + 1],
                                    scalar2=dg[:, col:col + 1],
                                    op0=mybir.AluOpType.is_equal,
                                    op1=mybir.AluOpType.mult,
                                )
                                nc.tensor.matmul(
                                    out=pss[w][:],
                                    lhsT=gb[:, lc * 128:(lc + 1) * 128],
                                    rhs=st[:],
                                    start=False,
                                    stop=(last_mm[w] == (c, t)),
                                )
                    for w in ws:
                        if layer == 1:
                            nc.scalar.activation(
                                out=h1T[:, w * 128:(w + 1) * 128], in_=pss[w][:],
                                func=mybir.ActivationFunctionType.Relu,
                                bias=wc[:, WC_B1:WC_B1 + 1])
                        else:
                            h2 = h2_pool.tile([128, 128], f32, tag="h2")
                            nc.scalar.activation(
                                out=h2[:], in_=pss[w][:],
                                func=mybir.ActivationFunctionType.Relu,
                                bias=wc[:, WC_B2:WC_B2 + 1])
                            psl = psL_pool.tile([C, 128], f32, space="PSUM", tag="psL")
                            nc.tensor.matmul(out=psl[:], lhsT=wc[:, WC_WO:WC_WO + C],
                                             rhs=h2[:], start=True, stop=True)
                            lg = lg_pool.tile([C, 128], f32, tag="lg")
                            nc.vector.tensor_scalar(
                                out=lg[:], in0=psl[:],
                                scalar1=wc[:C, WC_BO:WC_BO + 1], scalar2=None,
                                op0=mybir.AluOpType.add)
                            nc.sync.dma_start(
                                out=logitsT[:, w * 128:(w + 1) * 128], in_=lg[:])

            stages = os.environ.get("K_STAGES", "BCDE")
            # ---- layer 1
            if "C" in stages:
                aggregate(1, p1, lambda c: p1_writes_by_chunk[c])

            # ---- stage D: p2_own + slab AllGathers
            p2_writes_by_slab = {j: [] for j in range(NSLAB)}
            if "D" not in stages:
                ag_insts = []
            for mg in range(WPC // PJ if "D" in stages else 0):  # 14 groups of 7 windows
                w0 = mg * PJ
                p2s = p1s_pool.tile([128, PJ * 128], f32, tag="p1s")
                for k in range(PJ):
                    w = w0 + k
                    ps = psB_pool.tile([128, 128], f32, space="PSUM", tag="psB")
                    nc.tensor.matmul(out=ps[:], lhsT=h1T[:, w * 128:(w + 1) * 128],
                                     rhs=wc[:, WC_WL2:WC_WL2 + 128], start=True, stop=True)
                    nc.scalar.activation(out=p2s[:, k * 128:(k + 1) * 128], in_=ps[:],
                                         func=mybir.ActivationFunctionType.Copy)
                wi = nc.sync.dma_start(
                    out=p2own[w0 * 128:(w0 + PJ) * 128, :].rearrange("(k p) f -> p k f", p=128),
                    in_=p2s[:].rearrange("p (k f) -> p k f", f=128),
                )
                p2_writes_by_slab[(w0 * 128) // SLAB_ROWS].append(wi)

            ag_insts = []
            for j in range(NSLAB if "D" in stages else 0):
                ag = nc.gpsimd.collective_compute(
                    "AllGather",
                    mybir.AluOpType.bypass,
                    replica_groups=[list(range(NCORE))],
                    ins=[p2own[j * SLAB_ROWS:(j + 1) * SLAB_ROWS, :]],
                    outs=[p2full[j * SLAB_TOT:(j + 1) * SLAB_TOT, :]],
                )
                for wi in p2_writes_by_slab[j]:
                    add_dep_helper(ag.ins, wi.ins, True, "ag after p2 write")
                # dma_gather from Shared address space faults the device at scale;
                # bounce each slab into a Local internal tensor and gather from that.
                cp = nc.sync.dma_start(
                    out=p2loc[j * SLAB_TOT:(j + 1) * SLAB_TOT, :],
                    in_=p2full[j * SLAB_TOT:(j + 1) * SLAB_TOT, :],
                )
                add_dep_helper(cp.ins, ag.ins, True, "copy after ag")
                ag_insts.append(cp)

            # ---- layer 2
            if "E" in stages:
                e_src = p1 if os.environ.get("K_E_SRC") == "p1" else p2loc
                aggregate(2, e_src, lambda c: ag_insts)

    nc.finalize()
    return nc


_CACHED = {}


def kernel(**inputs) -> np.ndarray:
    from concourse.bass_utils import run_bass_kernel_spmd

    in_maps, plan = _prep(**{k: np.asarray(v) for k, v in inputs.items()})
    key = (plan["T_total"], plan["Twc"].tobytes(), tuple(tuple(c) for c in plan["calls"]))
    if key not in _CACHED:
        _CACHED[key] = _build(plan)
    nc = _CACHED[key]
    res = run_bass_kernel_spmd(nc, in_maps, list(range(NCORE)))
    out = np.concatenate([res.results[c]["logitsT"].T for c in range(NCORE)], axis=0)
    return np.ascontiguousarray(out[:N])

